# revision 1
# baseline (speedup 1.0000x reference)
"""Trainium2 Bass kernel for nn_LogicGatedSNN.

Computation (see reference):
    w       = (synapse_states > 50)                  # binary weights [8192, 8192]
    current = spike_input @ w.T                      # [8192]
    spikes  = (v_mem + current + noise >= v_th)      # [8192]
    S       = spikes.sum()
    v_mem'  = (v_mem - 0.5*S + current) * (1-spikes) * 0.5
    v_th'   = clip(v_th + (spikes - 0.1)*0.01, 0.2, 5.0)

Sharding: synapse_states row-wise (out_features) across 8 cores; each core
computes its 1024-row slice of current/spikes/v_mem/v_th locally, with one
AllReduce for the spikes.sum() inhibition term.

The key device-side trick: since spike_input s[i] is binary,
    w[o,i]*s[i] == (state[o,i] > 150 - 100*s[i])
(states lie in [40, 59], so the threshold 150 can never be exceeded). This
lets one fused DVE op (scalar_tensor_tensor with accum_out) do the masked
compare AND the row reduction in a single pass over the 32MB/core weight
slice, keeping the kernel DMA-bound (~358 GB/s/core HBM).
"""

import numpy as np

import concourse.bass as bass
import concourse.bacc as bacc
import concourse.tile as tile
import concourse.mybir as mybir
from concourse import bass_utils

N_CORES = 8
OUT_F = 8192
IN_F = 8192
R = OUT_F // N_CORES          # 1024 rows per core
P = 128                       # SBUF partitions
OC = R // P                   # 8 output tiles of 128 rows per core

F32 = mybir.dt.float32

# Filled by kernel() with the BassKernelResults of the last run (for test
# harness introspection: exec_time_ns etc).
LAST_RESULT = None

_CACHED_NC = None


def _build_nc():
    """Build the SPMD program (identical on all 8 cores)."""
    nc = bacc.Bacc(
        "TRN2", target_bir_lowering=False, debug=False, num_devices=N_CORES
    )

    states = nc.dram_tensor("states", [R, IN_F], F32, kind="ExternalInput")
    thr = nc.dram_tensor("thr", [1, IN_F], F32, kind="ExternalInput")
    v_mem_i = nc.dram_tensor("v_mem", [R], F32, kind="ExternalInput")
    v_th_i = nc.dram_tensor("v_th", [R], F32, kind="ExternalInput")
    noise_i = nc.dram_tensor("noise", [R], F32, kind="ExternalInput")

    spikes_o = nc.dram_tensor("spikes", [R], F32, kind="ExternalOutput")
    v_mem_o = nc.dram_tensor("v_mem_new", [R], F32, kind="ExternalOutput")
    v_th_o = nc.dram_tensor("v_th_new", [R], F32, kind="ExternalOutput")

    ALU = mybir.AluOpType

    # [1024] DRAM vector <-> [128, OC] SBUF tile with tile[p, a] = v[a*128 + p]
    def col_view(dram_t):
        return dram_t[:].rearrange("(a p) -> p a", p=P)

    with tile.TileContext(nc) as tc:
        with (
            tc.tile_pool(name="data", bufs=3) as data_pool,
            tc.tile_pool(name="aux", bufs=1) as aux,
            tc.tile_pool(name="dram", bufs=1, space="DRAM") as dram,
        ):
            # Broadcast per-column thresholds to all 128 partitions.
            thr_row = aux.tile([1, IN_F], F32)
            nc.scalar.dma_start(thr_row[:], thr[:, :])
            thr_b = aux.tile([P, IN_F], F32)
            nc.gpsimd.partition_broadcast(thr_b[:], thr_row[:])

            # Small per-core state vectors in [128, OC] layout.
            v_mem_sb = aux.tile([P, OC], F32)
            v_th_sb = aux.tile([P, OC], F32)
            noise_sb = aux.tile([P, OC], F32)
            nc.scalar.dma_start(v_mem_sb[:], col_view(v_mem_i))
            nc.scalar.dma_start(v_th_sb[:], col_view(v_th_i))
            nc.scalar.dma_start(noise_sb[:], col_view(noise_i))

            cur = aux.tile([P, OC], F32)     # current, one column per o-tile
            scratch = aux.tile([P, IN_F], F32)

            # Main loop: stream the 32MB weight slice, fused compare+reduce.
            for oc in range(OC):
                t = data_pool.tile([P, IN_F], F32, tag="w")
                nc.sync.dma_start(t[:], states[oc * P : (oc + 1) * P, :])
                # scratch = (t + 0) is_gt thr_b ; cur[:, oc] = sum(scratch)
                nc.vector.scalar_tensor_tensor(
                    out=scratch[:],
                    in0=t[:],
                    scalar=0.0,
                    in1=thr_b[:],
                    op0=ALU.add,
                    op1=ALU.is_gt,
                    accum_out=cur[:, oc : oc + 1],
                )

            # potential = (v_mem + current) + noise ; spikes = potential >= v_th
            pot = aux.tile([P, OC], F32)
            nc.vector.tensor_tensor(pot[:], v_mem_sb[:], cur[:], ALU.add)
            nc.vector.tensor_tensor(pot[:], pot[:], noise_sb[:], ALU.add)
            spikes_sb = aux.tile([P, OC], F32)
            nc.vector.tensor_tensor(spikes_sb[:], pot[:], v_th_sb[:], ALU.is_ge)
            nc.scalar.dma_start(col_view(spikes_o), spikes_sb[:])

            # Local spike count -> AllReduce across cores -> S broadcast.
            rowsum = aux.tile([P, 1], F32)
            nc.vector.tensor_reduce(
                rowsum[:], spikes_sb[:], axis=mybir.AxisListType.X, op=ALU.add
            )
            local_s = aux.tile([P, 1], F32)
            nc.gpsimd.partition_all_reduce(
                local_s[:], rowsum[:], channels=P, reduce_op=bass.bass_isa.ReduceOp.add
            )
            cc_in = dram.tile([1, 1], F32)
            cc_out = dram.tile([1, 1], F32)
            nc.scalar.dma_start(cc_in[:], local_s[0:1, 0:1])
            nc.gpsimd.collective_compute(
                "AllReduce",
                ALU.add,
                replica_groups=[list(range(N_CORES))],
                ins=[cc_in.opt()],
                outs=[cc_out.opt()],
            )
            s_row = aux.tile([1, 1], F32)
            nc.scalar.dma_start(s_row[:], cc_out[:])
            s_b = aux.tile([P, 1], F32)
            nc.gpsimd.partition_broadcast(s_b[:], s_row[:])

            # v_mem' = ((v_mem - 0.5*S) + current) * 0.5 * (1 - spikes)
            s_half = aux.tile([P, 1], F32)
            nc.vector.tensor_scalar_mul(s_half[:], s_b[:], 0.5)
            vm = aux.tile([P, OC], F32)
            nc.vector.tensor_scalar(
                out=vm[:], in0=v_mem_sb[:], scalar1=s_half[:], scalar2=None,
                op0=ALU.subtract,
            )
            nc.vector.tensor_tensor(vm[:], vm[:], cur[:], ALU.add)
            mask_neg = aux.tile([P, OC], F32)  # spikes - 1 == -(reset mask)
            nc.vector.tensor_scalar(
                out=mask_neg[:], in0=spikes_sb[:], scalar1=1.0, scalar2=None,
                op0=ALU.subtract,
            )
            # vm = (vm * -0.5) * (spikes - 1)  == (vm * 0.5) * (1 - spikes)
            nc.vector.scalar_tensor_tensor(
                out=vm[:], in0=vm[:], scalar=-0.5, in1=mask_neg[:],
                op0=ALU.mult, op1=ALU.mult,
            )
            nc.scalar.dma_start(col_view(v_mem_o), vm[:])

            # v_th' = clip(v_th + (spikes - 0.1) * 0.01, 0.2, 5.0)
            vt = aux.tile([P, OC], F32)
            nc.vector.tensor_scalar(
                out=vt[:], in0=spikes_sb[:], scalar1=0.1, scalar2=0.01,
                op0=ALU.subtract, op1=ALU.mult,
            )
            nc.vector.tensor_tensor(vt[:], vt[:], v_th_sb[:], ALU.add)
            nc.vector.tensor_scalar(
                out=vt[:], in0=vt[:], scalar1=0.2, scalar2=5.0,
                op0=ALU.max, op1=ALU.min,
            )
            nc.scalar.dma_start(col_view(v_th_o), vt[:])

    nc.compile()
    return nc


def kernel(spike_input, synapse_states, v_mem, v_th, noise):
    global LAST_RESULT, _CACHED_NC

    spike_input = np.ascontiguousarray(spike_input, dtype=np.float32)
    synapse_states = np.ascontiguousarray(synapse_states, dtype=np.float32)
    v_mem = np.ascontiguousarray(v_mem, dtype=np.float32)
    v_th = np.ascontiguousarray(v_th, dtype=np.float32)
    noise = np.ascontiguousarray(noise, dtype=np.float32)

    # w[o,i]*s[i] == (state[o,i] > thr[i]) with thr = 150 - 100*s  (s binary,
    # states in [40, 59])
    thr = (150.0 - 100.0 * spike_input.reshape(1, IN_F)).astype(np.float32)

    if _CACHED_NC is None:
        _CACHED_NC = _build_nc()
    nc = _CACHED_NC

    in_maps = []
    for c in range(N_CORES):
        sl = slice(c * R, (c + 1) * R)
        in_maps.append(
            {
                "states": synapse_states[sl],
                "thr": thr,
                "v_mem": v_mem[sl],
                "v_th": v_th[sl],
                "noise": noise[sl],
            }
        )

    res = bass_utils.run_bass_kernel_spmd(
        nc, in_maps, core_ids=list(range(N_CORES))
    )
    LAST_RESULT = res

    spikes = np.concatenate([res.results[c]["spikes"] for c in range(N_CORES)])
    v_mem_new = np.concatenate([res.results[c]["v_mem_new"] for c in range(N_CORES)])
    v_th_new = np.concatenate([res.results[c]["v_th_new"] for c in range(N_CORES)])
    return spikes, v_mem_new, v_th_new
